# revision 61
# baseline (speedup 1.0000x reference)
"""Trainium2 Bass kernel for BPPS model (LayerNorm -> per-species MLP -> segment sum).

Self-contained: hardcodes shapes from the problem spec.
  ps [200000, 512] f32, species_idx [200000] int, batch [200000] int (sorted),
  ln_gamma/ln_beta [512], W1 [4,512,256], W2 [4,256,256], W3 [4,256,1], W_comp [1,4].
Output: energies [2000, 1] f32.

Strategy: data-parallel over atoms on 8 NeuronCores (25000 atoms/core);
atoms are dealt to cores per species so every core runs one SPMD program.
Host does layout + LayerNorm + fp8 quantization; device does all GEMMs and
SiLUs; per-atom energies come back and the tiny segment-sum runs on host.

Device pipeline (per 8-tile group of 1024 atoms, [hidden_p, atoms] layout,
ACT-bound at ~93us/core of SiLU):
  L1: 32 fp8 DoubleRow matmuls (W1 stationary, X moving) -> P1S [128, 2, 1024]
      f32 (static 4-bank PSUM tile); ONE SiLU instr ([128, 2048] PSUM -> fp8
      SBUF) amortizing the ~185ns per-instruction ACT access overhead;
  L2: 16 fp8 DR matmuls (W2 stationary, H1 moving, no transpose needed) ->
      P2S (4 banks); SiLU -> H2 bf16;
  W3: 16 near-free bf16 matmuls with [128, 1] outputs into a dead corner of
      the idle psum slot; DVE copies to an SBUF energy row, one DMA out.
The two silus of adjacent groups interleave on ACT with zero steady-state
stalls; the last group's L2 retargets P1S (dead by then) to avoid the
drain bubble. No transposes, no Gram/rsqrt stats, no one-hot segment matmul.
"""

import sys

sys.path.insert(0, "/opt/trn_rl_repo")

import numpy as np
import ml_dtypes

BF16 = ml_dtypes.bfloat16
FP8 = ml_dtypes.float8_e4m3

# Problem constants
N_ATOMS = 200000
D_IN = 512
HIDDEN = 256
N_SPECIES = 4
N_STRUCT = 2000
AVG_N_ATOMS = 60.0
E_SCALE = 1.0
LN_EPS = 1e-5

N_CORES = 8
APC = N_ATOMS // N_CORES        # 25000 atoms per core
P = 128
FP8_MAX = 224.0                 # headroom under trn e4m3 max (240)


def _q8(x):
    return np.clip(x, -FP8_MAX, FP8_MAX).astype(FP8)


# ----------------------------------------------------------------------------
# Host-side layout preparation
# ----------------------------------------------------------------------------

def host_prep(ps, ln_gamma, ln_beta, W1, W2, W3, W_comp, species_idx, batch):
    ps = np.asarray(ps, dtype=np.float32)
    species_idx = np.asarray(species_idx).astype(np.int64)
    batch = np.asarray(batch).astype(np.int64)
    ln_gamma = np.asarray(ln_gamma, dtype=np.float32)
    ln_beta = np.asarray(ln_beta, dtype=np.float32)
    W1 = np.asarray(W1, dtype=np.float32)
    W2 = np.asarray(W2, dtype=np.float32)
    W3f = np.asarray(W3, dtype=np.float32)

    # exact LayerNorm on host (host already touches every element to quantize)
    mu = ps.mean(axis=1, keepdims=True)
    var = ps.var(axis=1, keepdims=True)
    x = (ps - mu) * (1.0 / np.sqrt(var + LN_EPS)) * ln_gamma + ln_beta

    sX = FP8_MAX / float(np.abs(x).max())
    sW1 = FP8_MAX / float(np.abs(W1).max())
    sW2 = FP8_MAX / float(np.abs(W2).max())
    scale1 = 1.0 / (sX * sW1)
    scale2 = 1.0 / sW2

    # Global species sort, atoms dealt per-species evenly across cores: any
    # atom may go to any core (segment-sum happens on host), so per-core
    # per-species counts are equal +-1 and the shared SPMD tile map is minimal.
    order_g = np.argsort(species_idx, kind="stable")
    n_sp = np.bincount(species_idx, minlength=N_SPECIES)
    pool_off = np.cumsum([0] + list(n_sp))
    cnt_cs = np.zeros((N_CORES, N_SPECIES), dtype=np.int64)
    for s in range(N_SPECIES):
        q, r = divmod(int(n_sp[s]), N_CORES)
        cnt_cs[:, s] = q
        cnt_cs[:r, s] += 1
    T_sp = [int(np.ceil(cnt_cs[:, s].max() / P)) for s in range(N_SPECIES)]
    T = sum(T_sp)
    species_map = []
    for s in range(N_SPECIES):
        species_map += [s] * T_sp[s]
    offs = np.cumsum([0] + T_sp)     # tile offset of each species block

    xq = _q8(x * sX)
    xt_all = np.zeros((N_CORES, P, T, 2, 2, P), dtype=FP8)
    perms = np.full((N_CORES, T * P), -1, dtype=np.int64)
    for c in range(N_CORES):
        xpad = np.zeros((T * P, D_IN), dtype=FP8)
        for s in range(N_SPECIES):
            cnt = int(cnt_cs[c, s])
            p0 = int(pool_off[s]) + int(cnt_cs[:c, s].sum())
            sel = order_g[p0:p0 + cnt]
            s0 = int(offs[s]) * P
            xpad[s0:s0 + cnt] = xq[sel]
            perms[c, s0:s0 + cnt] = sel
        # feature f = 256*c + 128*j + p : [t, a, c, j, p] -> [p, t, c, j, a]
        xt_all[c] = xpad.reshape(T, P, 2, 2, P).transpose(4, 0, 2, 3, 1)

    # per-(structure, species) atom counts for the composition term
    counts_ss = np.zeros((N_STRUCT, N_SPECIES), dtype=np.float64)
    np.add.at(counts_ss, (batch, species_idx), 1.0)

    # Weights, [hidden_p, *] stationary layouts.
    # w1r[s, c, m1] = [p, j, m] fp8 from W1[f= 256c+128j+p, h= 128*m1+m]
    w1r = _q8((W1 * sW1).reshape(N_SPECIES, 2, 2, P, 2, P)
              .transpose(0, 1, 4, 3, 2, 5))          # [s, c, m1, p, j, m]
    # w2r[s, m1] = [p, jj, m] from W2[h1= 128*jj+p, h2= 128*m1+m]
    w2r = _q8((W2 * sW2).reshape(N_SPECIES, 2, P, 2, P)
              .transpose(0, 3, 2, 1, 4))             # [s, m1, p, jj, m]
    # w3b[s, m1] = [p] bf16 from W3[h2 = 128*m1+p]
    w3b = W3f[:, :, 0].reshape(N_SPECIES, 2, P).astype(BF16)  # [s, m1, p]

    # one per-partition constant blob (single DMA):
    # [0:4096)    w1r  [s, c, m1][j, m] fp8
    # [4096:6144) w2r  [s, m1][jj, m] fp8
    # [6144:6160) w3b  [s, m1] bf16
    wb = np.zeros((P, 6160), dtype=np.uint8)
    wb[:, 0:4096] = w1r.transpose(3, 0, 1, 2, 4, 5).reshape(P, 4096).view(np.uint8)
    wb[:, 4096:6144] = w2r.transpose(2, 0, 1, 3, 4).reshape(P, 2048).view(np.uint8)
    wb[:, 6144:6160] = np.ascontiguousarray(
        w3b.transpose(2, 0, 1)).reshape(P, 8).view(np.uint8)
    wb = wb.view(FP8)

    in_maps = []
    for c in range(N_CORES):
        in_maps.append({
            "xt": np.ascontiguousarray(xt_all[c]),
            "wb": wb,
        })
    meta = dict(T=T, species_map=tuple(species_map), perms=perms,
                counts_ss=counts_ss, scale1=scale1, scale2=scale2)
    return in_maps, meta


# ----------------------------------------------------------------------------
# Device program
# ----------------------------------------------------------------------------

def build_program(T, species_map, scale1, scale2, bufs=None):
    import concourse.bacc as bacc
    import concourse.tile as tile
    from concourse import mybir

    B = {"depth": 3, "gs": 8, "h1": 3, "h2": 3, "prefix": ()}
    B.update(bufs or {})

    f32 = mybir.dt.float32
    bf16 = mybir.dt.bfloat16
    fp8 = mybir.dt.float8e4
    DR = mybir.MatmulPerfMode.DoubleRow
    SILU = mybir.ActivationFunctionType.Silu

    GS = B["gs"]                      # max tiles per group (8 => 4-bank psum tiles)
    DEPTH = B["depth"]
    EO = GS * P - 8                   # e scratch corner in a psum slot's last bank

    bounds = []
    t0 = 0
    for sz in B["prefix"]:
        if t0 + sz <= T:
            bounds.append((t0, t0 + sz))
            t0 += sz
    while t0 < T:
        t1 = min(t0 + GS, T)
        bounds.append((t0, t1))
        t0 = t1
    G = len(bounds)

    nc = bacc.Bacc("TRN2", target_bir_lowering=False, debug=False,
                   num_devices=N_CORES)
    xt_d = nc.dram_tensor("xt", [P, T, 2, 2, P], fp8, kind="ExternalInput")
    wb_d = nc.dram_tensor("wb", [P, 6160], fp8, kind="ExternalInput")
    out_d = nc.dram_tensor("e_out", [P, T], f32, kind="ExternalOutput")

    from contextlib import ExitStack
    with tile.TileContext(nc, trace_sim=False) as tc:
        with ExitStack() as ctx:
            singles = ctx.enter_context(tc.tile_pool(name="singles", bufs=1))
            xt_pool = ctx.enter_context(tc.tile_pool(name="xt", bufs=DEPTH + 1))
            h1_pool = ctx.enter_context(tc.tile_pool(name="h1", bufs=B["h1"]))
            h2_pool = ctx.enter_context(tc.tile_pool(name="h2", bufs=B["h2"]))
            psum_s = ctx.enter_context(tc.tile_pool(name="ps", bufs=1, space="PSUM"))
            # static psum tiles; subtile dependency tracking orders reuse
            P1S = psum_s.tile([P, 2, GS * P], f32, tag="p1s")
            P2S = psum_s.tile([P, 2, GS * P], f32, tag="p2s")

            # dummy first activation: hoists the Silu table load to t=0 on the
            # otherwise-idle ACT queue (it would otherwise land right before
            # the first real silu, on the critical path)
            DUM = singles.tile([P, 1], f32)
            nc.scalar.activation(DUM[:], DUM[:], SILU)

            WB = singles.tile([P, 6160], fp8)
            # species-0 W1 chunk first on the SP queue, ahead of the xt
            # prefetches, so L1(0) starts as early as possible
            nc.sync.dma_start(WB[:, 0:1024], wb_d.ap()[:, 0:1024])
            E = singles.tile([P, T], f32)

            def w1_ap(s, c, m1):     # [p, j, m] fp8 stationary chunk
                o = ((s * 2 + c) * 2 + m1) * 256
                return WB[:, o:o + 256].rearrange("p (j m) -> p j m", j=2)

            def w2_ap(s, m1):        # [p, jj, m] fp8
                o = 4096 + (s * 2 + m1) * 256
                return WB[:, o:o + 256].rearrange("p (jj m) -> p jj m", jj=2)

            def w3_ap(s, m1):        # [p, 1] bf16
                o = 6144 + (s * 2 + m1) * 2
                return WB[:, o:o + 2].bitcast(bf16)

            xts = {}

            def dma_in(g):
                if g >= G:
                    return
                lo, hi = bounds[g]
                tl = hi - lo
                XT = xt_pool.tile([P, GS, 2, 2, P], fp8)
                if g == 0:
                    # split so the first tiles' compute starts early
                    for i, i1 in ((0, 2), (2, tl)):
                        if min(i1, tl) > i:
                            nc.sync.dma_start(XT[:, i:min(i1, tl)],
                                              xt_d.ap()[:, lo + i:lo + min(i1, tl)])
                else:
                    nc.sync.dma_start(XT[:, 0:tl], xt_d.ap()[:, lo:hi])
                xts[g] = XT

            for a in range(DEPTH):
                dma_in(a)
            nc.sync.dma_start(WB[:, 1024:4096], wb_d.ap()[:, 1024:4096])
            nc.sync.dma_start(WB[:, 4096:6160], wb_d.ap()[:, 4096:6160])

            def w3_group(g2, dst):
                # per-atom energies of group g2 into a psum corner + E copy
                H2 = h2s.pop(g2)
                lo, hi2 = bounds[g2]
                tl2 = hi2 - lo
                for i in range(tl2):
                    s = species_map[lo + i]
                    for m1 in range(2):
                        nc.tensor.matmul(
                            dst[:, 1, EO + i:EO + i + 1],
                            H2[:, m1, P * i:P * (i + 1)], w3_ap(s, m1),
                            start=(m1 == 0), stop=(m1 == 1))
                nc.vector.tensor_copy(E[:, lo:lo + tl2], dst[:, 1, EO:EO + tl2])
                return lo, tl2

            def l1_tiles(g, i0, i1):
                XT = xts[g]
                for i in range(i0, i1):
                    s = species_map[bounds[g][0] + i]
                    for m1 in range(2):
                        for c in range(2):
                            nc.tensor.matmul(
                                P1S[:, m1, P * i:P * (i + 1)],
                                w1_ap(s, c, m1), XT[:, i, c],
                                start=(c == 0), stop=(c == 1),
                                perf_mode=DR)

            h1s, h2s = {}, {}
            for g in range(G + 2):
                if g < G:
                    dma_in(g + DEPTH)
                    if g == 0:
                        # interleave: deps are per-engine emission counters,
                        # so the first silu piece must be emitted before the
                        # later tiles' matmuls to start early
                        tl = bounds[0][1]
                        h = min(2, tl)
                        H1 = h1_pool.tile([P, 2, GS * P], fp8)
                        l1_tiles(0, 0, h)
                        nc.scalar.activation(H1[:, :, 0:P * h],
                                             P1S[:, :, 0:P * h],
                                             SILU, scale=scale1)
                        if tl > h:
                            l1_tiles(0, h, tl)
                            nc.scalar.activation(H1[:, :, P * h:P * tl],
                                                 P1S[:, :, P * h:P * tl],
                                                 SILU, scale=scale1)
                        h1s[0] = H1
                        xts.pop(0)
                    elif g != 1:
                        # g==1's L1 is emitted after L2(0)/silu2(0) below, so
                        # silu2(0) isn't stuck behind L1(1) in the PE queue
                        l1_tiles(g, 0, bounds[g][1] - bounds[g][0])
                        xts.pop(g)
                if 2 <= g < G:
                    # W3 of the group silu2'd one ACT slot ago; e lands in a
                    # dead corner of the p2 slot (idle between silu2(g-2)'s
                    # read and L2(g-1)'s write). Its dependency (silu2(g-2))
                    # is long done, so this never delays the ACT queue.
                    w3_group(g - 2, P2S)
                if 1 <= g <= G:
                    tl = bounds[g - 1][1] - bounds[g - 1][0]
                    H1p = h1s.pop(g - 1)
                    H2 = h2_pool.tile([P, 2, GS * P], bf16)
                    # last group: write P1S (dead once s1(G-1) has read it) so
                    # L2 runs during silu2(G-2) instead of after its drain
                    PL2 = P2S if g < G else P1S
                    for i in range(tl):
                        s = species_map[bounds[g - 1][0] + i]
                        for m1 in range(2):
                            nc.tensor.matmul(
                                PL2[:, m1, P * i:P * (i + 1)],
                                w2_ap(s, m1), H1p[:, :, P * i:P * (i + 1)],
                                start=True, stop=True, perf_mode=DR)
                    nc.scalar.activation(H2[:, :, 0:P * tl],
                                         PL2[:, :, 0:P * tl],
                                         SILU, scale=scale2)
                    h2s[g - 1] = H2
                if g == 1 and g < G:
                    l1_tiles(1, 0, bounds[1][1] - bounds[1][0])
                    xts.pop(1)
                if 1 <= g < G:
                    tl = bounds[g][1] - bounds[g][0]
                    H1 = h1_pool.tile([P, 2, GS * P], fp8)
                    nc.scalar.activation(H1[:, :, 0:P * tl],
                                         P1S[:, :, 0:P * tl],
                                         SILU, scale=scale1)
                    h1s[g] = H1
                if g == G and g >= 2:
                    # P2S is dead now (last L2 went to P1S): W3(G-2)'s corner
                    # write only waits silu2(G-2), running during silu2(G-1)
                    lo, tl2 = w3_group(g - 2, P2S)
                    nc.sync.dma_start(out_d.ap()[:, 0:lo + tl2],
                                      E[:, 0:lo + tl2])
                if g == G + 1 and g >= 3:
                    lo, tl2 = w3_group(g - 2, P1S)
                    nc.sync.dma_start(out_d.ap()[:, lo:lo + tl2],
                                      E[:, lo:lo + tl2])

    nc.compile()
    return nc


# ----------------------------------------------------------------------------
# Aggregation
# ----------------------------------------------------------------------------

def aggregate(results, meta, batch, species_idx, W_comp):
    W_comp = np.asarray(W_comp, dtype=np.float64)
    batch = np.asarray(batch).astype(np.int64)
    perms = meta["perms"]
    E = np.zeros(N_STRUCT, dtype=np.float64)
    for c in range(N_CORES):
        e = np.asarray(results[c]["e_out"], dtype=np.float64)  # [128, T]
        e_slots = e.T.reshape(-1)                 # slot = t*128 + lane
        pm = perms[c]
        valid = pm >= 0
        E += np.bincount(batch[pm[valid]], weights=e_slots[valid],
                         minlength=N_STRUCT)
    energies = (E / AVG_N_ATOMS)[:, None] * E_SCALE \
        + meta["counts_ss"] @ W_comp.T
    return energies.astype(np.float32)


# ----------------------------------------------------------------------------
# Entry point
# ----------------------------------------------------------------------------

_PROGRAM_CACHE = {}


def kernel(ps, ln_gamma, ln_beta, W1, W2, W3, W_comp, species_idx, batch):
    from concourse import bass_utils

    in_maps, meta = host_prep(ps, ln_gamma, ln_beta, W1, W2, W3, W_comp,
                              species_idx, batch)
    key = (meta["T"], meta["species_map"], meta["scale1"], meta["scale2"])
    if key not in _PROGRAM_CACHE:
        _PROGRAM_CACHE[key] = build_program(meta["T"], meta["species_map"],
                                            meta["scale1"], meta["scale2"])
    nc = _PROGRAM_CACHE[key]
    res = bass_utils.run_bass_kernel_spmd(nc, in_maps,
                                          core_ids=list(range(N_CORES)))
    return aggregate(res.results, meta, batch, species_idx, W_comp)


# revision 64
# speedup vs baseline: 1.0056x; 1.0056x over previous
"""Trainium2 Bass kernel for BPPS model (LayerNorm -> per-species MLP -> segment sum).

Self-contained: hardcodes shapes from the problem spec.
  ps [200000, 512] f32, species_idx [200000] int, batch [200000] int (sorted),
  ln_gamma/ln_beta [512], W1 [4,512,256], W2 [4,256,256], W3 [4,256,1], W_comp [1,4].
Output: energies [2000, 1] f32.

Strategy: data-parallel over atoms on 8 NeuronCores (25000 atoms/core);
atoms are dealt to cores per species so every core runs one SPMD program.
Host does layout + LayerNorm + fp8 quantization; device does all GEMMs and
SiLUs; per-atom energies come back and the tiny segment-sum runs on host.

Device pipeline (per 8-tile group of 1024 atoms, [hidden_p, atoms] layout,
ACT-bound at ~93us/core of SiLU):
  L1: 32 fp8 DoubleRow matmuls (W1 stationary, X moving) -> P1S [128, 2, 1024]
      f32 (static 4-bank PSUM tile); ONE SiLU instr ([128, 2048] PSUM -> fp8
      SBUF) amortizing the ~185ns per-instruction ACT access overhead;
  L2: 16 fp8 DR matmuls (W2 stationary, H1 moving, no transpose needed) ->
      P2S (4 banks); SiLU -> H2 bf16;
  W3: 16 near-free bf16 matmuls with [128, 1] outputs into a dead corner of
      the idle psum slot; DVE copies to an SBUF energy row, one DMA out.
The two silus of adjacent groups interleave on ACT with zero steady-state
stalls; the last group's L2 retargets P1S (dead by then) to avoid the
drain bubble. No transposes, no Gram/rsqrt stats, no one-hot segment matmul.
"""

import sys

sys.path.insert(0, "/opt/trn_rl_repo")

import numpy as np
import ml_dtypes

BF16 = ml_dtypes.bfloat16
FP8 = ml_dtypes.float8_e4m3

# Problem constants
N_ATOMS = 200000
D_IN = 512
HIDDEN = 256
N_SPECIES = 4
N_STRUCT = 2000
AVG_N_ATOMS = 60.0
E_SCALE = 1.0
LN_EPS = 1e-5

N_CORES = 8
APC = N_ATOMS // N_CORES        # 25000 atoms per core
P = 128
FP8_MAX = 224.0                 # headroom under trn e4m3 max (240)


def _q8(x):
    return np.clip(x, -FP8_MAX, FP8_MAX).astype(FP8)


# ----------------------------------------------------------------------------
# Host-side layout preparation
# ----------------------------------------------------------------------------

def host_prep(ps, ln_gamma, ln_beta, W1, W2, W3, W_comp, species_idx, batch):
    ps = np.asarray(ps, dtype=np.float32)
    species_idx = np.asarray(species_idx).astype(np.int64)
    batch = np.asarray(batch).astype(np.int64)
    ln_gamma = np.asarray(ln_gamma, dtype=np.float32)
    ln_beta = np.asarray(ln_beta, dtype=np.float32)
    W1 = np.asarray(W1, dtype=np.float32)
    W2 = np.asarray(W2, dtype=np.float32)
    W3f = np.asarray(W3, dtype=np.float32)

    # exact LayerNorm on host (host already touches every element to quantize)
    mu = ps.mean(axis=1, keepdims=True)
    var = ps.var(axis=1, keepdims=True)
    x = (ps - mu) * (1.0 / np.sqrt(var + LN_EPS)) * ln_gamma + ln_beta

    sX = FP8_MAX / float(np.abs(x).max())
    sW1 = FP8_MAX / float(np.abs(W1).max())
    sW2 = FP8_MAX / float(np.abs(W2).max())
    scale1 = 1.0 / (sX * sW1)
    scale2 = 1.0 / sW2

    # Global species sort, atoms dealt per-species evenly across cores: any
    # atom may go to any core (segment-sum happens on host), so per-core
    # per-species counts are equal +-1 and the shared SPMD tile map is minimal.
    order_g = np.argsort(species_idx, kind="stable")
    n_sp = np.bincount(species_idx, minlength=N_SPECIES)
    pool_off = np.cumsum([0] + list(n_sp))
    cnt_cs = np.zeros((N_CORES, N_SPECIES), dtype=np.int64)
    for s in range(N_SPECIES):
        q, r = divmod(int(n_sp[s]), N_CORES)
        cnt_cs[:, s] = q
        cnt_cs[:r, s] += 1
    T_sp = [int(np.ceil(cnt_cs[:, s].max() / P)) for s in range(N_SPECIES)]
    T = sum(T_sp)
    species_map = []
    for s in range(N_SPECIES):
        species_map += [s] * T_sp[s]
    offs = np.cumsum([0] + T_sp)     # tile offset of each species block

    xq = _q8(x * sX)
    xt_all = np.zeros((N_CORES, P, T, 2, 2, P), dtype=FP8)
    perms = np.full((N_CORES, T * P), -1, dtype=np.int64)
    for c in range(N_CORES):
        xpad = np.zeros((T * P, D_IN), dtype=FP8)
        for s in range(N_SPECIES):
            cnt = int(cnt_cs[c, s])
            p0 = int(pool_off[s]) + int(cnt_cs[:c, s].sum())
            sel = order_g[p0:p0 + cnt]
            s0 = int(offs[s]) * P
            xpad[s0:s0 + cnt] = xq[sel]
            perms[c, s0:s0 + cnt] = sel
        # feature f = 256*c + 128*j + p : [t, a, c, j, p] -> [p, t, c, j, a]
        xt_all[c] = xpad.reshape(T, P, 2, 2, P).transpose(4, 0, 2, 3, 1)

    # per-(structure, species) atom counts for the composition term
    counts_ss = np.zeros((N_STRUCT, N_SPECIES), dtype=np.float64)
    np.add.at(counts_ss, (batch, species_idx), 1.0)

    # Weights, [hidden_p, *] stationary layouts.
    # w1r[s, c, m1] = [p, j, m] fp8 from W1[f= 256c+128j+p, h= 128*m1+m]
    w1r = _q8((W1 * sW1).reshape(N_SPECIES, 2, 2, P, 2, P)
              .transpose(0, 1, 4, 3, 2, 5))          # [s, c, m1, p, j, m]
    # w2r[s, m1] = [p, jj, m] from W2[h1= 128*jj+p, h2= 128*m1+m]
    w2r = _q8((W2 * sW2).reshape(N_SPECIES, 2, P, 2, P)
              .transpose(0, 3, 2, 1, 4))             # [s, m1, p, jj, m]
    # w3b[s, m1] = [p] bf16 from W3[h2 = 128*m1+p]
    w3b = W3f[:, :, 0].reshape(N_SPECIES, 2, P).astype(BF16)  # [s, m1, p]

    # one per-partition constant blob (single DMA):
    # [0:4096)    w1r  [s, c, m1][j, m] fp8
    # [4096:6144) w2r  [s, m1][jj, m] fp8
    # [6144:6160) w3b  [s, m1] bf16
    wb = np.zeros((P, 6160), dtype=np.uint8)
    wb[:, 0:4096] = w1r.transpose(3, 0, 1, 2, 4, 5).reshape(P, 4096).view(np.uint8)
    wb[:, 4096:6144] = w2r.transpose(2, 0, 1, 3, 4).reshape(P, 2048).view(np.uint8)
    wb[:, 6144:6160] = np.ascontiguousarray(
        w3b.transpose(2, 0, 1)).reshape(P, 8).view(np.uint8)
    wb = wb.view(FP8)

    in_maps = []
    for c in range(N_CORES):
        in_maps.append({
            "xt": np.ascontiguousarray(xt_all[c]),
            "wb": wb,
        })
    meta = dict(T=T, species_map=tuple(species_map), perms=perms,
                counts_ss=counts_ss, scale1=scale1, scale2=scale2)
    return in_maps, meta


# ----------------------------------------------------------------------------
# Device program
# ----------------------------------------------------------------------------

def build_program(T, species_map, scale1, scale2, bufs=None):
    import concourse.bacc as bacc
    import concourse.tile as tile
    from concourse import mybir

    B = {"depth": 3, "gs": 8, "h1": 3, "h2": 3, "prefix": ()}
    B.update(bufs or {})

    f32 = mybir.dt.float32
    bf16 = mybir.dt.bfloat16
    fp8 = mybir.dt.float8e4
    DR = mybir.MatmulPerfMode.DoubleRow
    SILU = mybir.ActivationFunctionType.Silu

    GS = B["gs"]                      # max tiles per group (8 => 4-bank psum tiles)
    DEPTH = B["depth"]
    EO = GS * P - 8                   # e scratch corner in a psum slot's last bank

    bounds = []
    t0 = 0
    for sz in B["prefix"]:
        if t0 + sz <= T:
            bounds.append((t0, t0 + sz))
            t0 += sz
    while t0 < T:
        t1 = min(t0 + GS, T)
        bounds.append((t0, t1))
        t0 = t1
    G = len(bounds)

    nc = bacc.Bacc("TRN2", target_bir_lowering=False, debug=False,
                   num_devices=N_CORES)
    xt_d = nc.dram_tensor("xt", [P, T, 2, 2, P], fp8, kind="ExternalInput")
    wb_d = nc.dram_tensor("wb", [P, 6160], fp8, kind="ExternalInput")
    out_d = nc.dram_tensor("e_out", [P, T], f32, kind="ExternalOutput")

    from contextlib import ExitStack
    with tile.TileContext(nc, trace_sim=False) as tc:
        with ExitStack() as ctx:
            singles = ctx.enter_context(tc.tile_pool(name="singles", bufs=1))
            xt_pool = ctx.enter_context(tc.tile_pool(name="xt", bufs=DEPTH + 1))
            h1_pool = ctx.enter_context(tc.tile_pool(name="h1", bufs=B["h1"]))
            h2_pool = ctx.enter_context(tc.tile_pool(name="h2", bufs=B["h2"]))
            psum_s = ctx.enter_context(tc.tile_pool(name="ps", bufs=1, space="PSUM"))
            # static psum tiles; subtile dependency tracking orders reuse
            P1S = psum_s.tile([P, 2, GS * P], f32, tag="p1s")
            P2S = psum_s.tile([P, 2, GS * P], f32, tag="p2s")

            # dummy first activation: hoists the Silu table load to t=0 on the
            # otherwise-idle ACT queue (it would otherwise land right before
            # the first real silu, on the critical path)
            DUM = singles.tile([P, 1], f32)
            nc.scalar.activation(DUM[:], DUM[:], SILU)

            WB = singles.tile([P, 6160], fp8)
            # species-0 W1 chunk first on the SP queue, ahead of the xt
            # prefetches, so L1(0) starts as early as possible
            nc.sync.dma_start(WB[:, 0:1024], wb_d.ap()[:, 0:1024])
            E = singles.tile([P, T], f32)

            def w1_ap(s, c, m1):     # [p, j, m] fp8 stationary chunk
                o = ((s * 2 + c) * 2 + m1) * 256
                return WB[:, o:o + 256].rearrange("p (j m) -> p j m", j=2)

            def w2_ap(s, m1):        # [p, jj, m] fp8
                o = 4096 + (s * 2 + m1) * 256
                return WB[:, o:o + 256].rearrange("p (jj m) -> p jj m", jj=2)

            def w3_ap(s, m1):        # [p, 1] bf16
                o = 6144 + (s * 2 + m1) * 2
                return WB[:, o:o + 2].bitcast(bf16)

            xts = {}

            def dma_in(g):
                if g >= G:
                    return
                lo, hi = bounds[g]
                tl = hi - lo
                XT = xt_pool.tile([P, GS, 2, 2, P], fp8)
                if g == 0:
                    # split so the first tiles' compute starts early
                    for i, i1 in ((0, 2), (2, tl)):
                        if min(i1, tl) > i:
                            nc.sync.dma_start(XT[:, i:min(i1, tl)],
                                              xt_d.ap()[:, lo + i:lo + min(i1, tl)])
                else:
                    nc.sync.dma_start(XT[:, 0:tl], xt_d.ap()[:, lo:hi])
                xts[g] = XT

            for a in range(DEPTH):
                dma_in(a)
            nc.sync.dma_start(WB[:, 1024:4096], wb_d.ap()[:, 1024:4096])
            nc.sync.dma_start(WB[:, 4096:6160], wb_d.ap()[:, 4096:6160])

            def w3_group(g2, dst):
                # per-atom energies of group g2 into a psum corner + E copy
                H2 = h2s.pop(g2)
                lo, hi2 = bounds[g2]
                tl2 = hi2 - lo
                for i in range(tl2):
                    s = species_map[lo + i]
                    for m1 in range(2):
                        nc.tensor.matmul(
                            dst[:, 1, EO + i:EO + i + 1],
                            H2[:, m1, P * i:P * (i + 1)], w3_ap(s, m1),
                            start=(m1 == 0), stop=(m1 == 1))
                nc.vector.tensor_copy(E[:, lo:lo + tl2], dst[:, 1, EO:EO + tl2])
                return lo, tl2

            def l1_tiles(g, i0, i1):
                XT = xts[g]
                for i in range(i0, i1):
                    s = species_map[bounds[g][0] + i]
                    for m1 in range(2):
                        for c in range(2):
                            nc.tensor.matmul(
                                P1S[:, m1, P * i:P * (i + 1)],
                                w1_ap(s, c, m1), XT[:, i, c],
                                start=(c == 0), stop=(c == 1),
                                perf_mode=DR)

            h1s, h2s = {}, {}
            for g in range(G + 2):
                if g < G:
                    dma_in(g + DEPTH)
                    if g == 0:
                        # interleave: deps are per-engine emission counters,
                        # so the first silu piece must be emitted before the
                        # later tiles' matmuls to start early
                        tl = bounds[0][1]
                        h = min(2, tl)
                        H1 = h1_pool.tile([P, 2, GS * P], fp8)
                        l1_tiles(0, 0, h)
                        nc.scalar.activation(H1[:, :, 0:P * h],
                                             P1S[:, :, 0:P * h],
                                             SILU, scale=scale1)
                        if tl > h:
                            l1_tiles(0, h, tl)
                            nc.scalar.activation(H1[:, :, P * h:P * tl],
                                                 P1S[:, :, P * h:P * tl],
                                                 SILU, scale=scale1)
                        h1s[0] = H1
                        xts.pop(0)
                    elif g != 1:
                        # g==1's L1 is emitted after L2(0)/silu2(0) below, so
                        # silu2(0) isn't stuck behind L1(1) in the PE queue
                        l1_tiles(g, 0, bounds[g][1] - bounds[g][0])
                        xts.pop(g)
                late_w3 = (g == G - 1 and g >= 2
                           and bounds[-1][1] - bounds[-1][0] < GS)
                if 2 <= g < G and not late_w3:
                    # W3 of the group silu2'd one ACT slot ago; e lands in a
                    # dead corner of the p2 slot (idle between silu2(g-2)'s
                    # read and L2(g-1)'s write). Its dependency (silu2(g-2))
                    # is long done, so this never delays the ACT queue.
                    w3_group(g - 2, P2S)
                if 1 <= g <= G:
                    tl = bounds[g - 1][1] - bounds[g - 1][0]
                    H1p = h1s.pop(g - 1)
                    H2 = h2_pool.tile([P, 2, GS * P], bf16)
                    # last group: write P1S (dead once s1(G-1) has read it) so
                    # L2 runs during silu2(G-2) instead of after its drain
                    PL2 = P2S if g < G else P1S
                    for i in range(tl):
                        s = species_map[bounds[g - 1][0] + i]
                        for m1 in range(2):
                            nc.tensor.matmul(
                                PL2[:, m1, P * i:P * (i + 1)],
                                w2_ap(s, m1), H1p[:, :, P * i:P * (i + 1)],
                                start=True, stop=True, perf_mode=DR)
                    nc.scalar.activation(H2[:, :, 0:P * tl],
                                         PL2[:, :, 0:P * tl],
                                         SILU, scale=scale2)
                    h2s[g - 1] = H2
                if g == 1 and g < G:
                    l1_tiles(1, 0, bounds[1][1] - bounds[1][0])
                    xts.pop(1)
                if 1 <= g < G:
                    tl = bounds[g][1] - bounds[g][0]
                    H1 = h1_pool.tile([P, 2, GS * P], fp8)
                    nc.scalar.activation(H1[:, :, 0:P * tl],
                                         P1S[:, :, 0:P * tl],
                                         SILU, scale=scale1)
                    h1s[g] = H1
                if late_w3:
                    # partial last group never touches the psum corner, so at
                    # iter G-1 W3(G-3) can run after L2/silu2 into P1S's
                    # corner; L2(G-2) then follows silu2(G-3) directly and
                    # its chain fits the narrow partial-group ACT slot
                    w3_group(g - 2, P1S)
                if g == G and g >= 2:
                    # P2S is dead now (last L2 went to P1S): W3(G-2)'s corner
                    # write only waits silu2(G-2), running during silu2(G-1)
                    lo, tl2 = w3_group(g - 2, P2S)
                    nc.sync.dma_start(out_d.ap()[:, 0:lo + tl2],
                                      E[:, 0:lo + tl2])
                if g == G + 1 and g >= 3:
                    lo, tl2 = w3_group(g - 2, P1S)
                    nc.sync.dma_start(out_d.ap()[:, lo:lo + tl2],
                                      E[:, lo:lo + tl2])

    nc.compile()
    return nc


# ----------------------------------------------------------------------------
# Aggregation
# ----------------------------------------------------------------------------

def aggregate(results, meta, batch, species_idx, W_comp):
    W_comp = np.asarray(W_comp, dtype=np.float64)
    batch = np.asarray(batch).astype(np.int64)
    perms = meta["perms"]
    E = np.zeros(N_STRUCT, dtype=np.float64)
    for c in range(N_CORES):
        e = np.asarray(results[c]["e_out"], dtype=np.float64)  # [128, T]
        e_slots = e.T.reshape(-1)                 # slot = t*128 + lane
        pm = perms[c]
        valid = pm >= 0
        E += np.bincount(batch[pm[valid]], weights=e_slots[valid],
                         minlength=N_STRUCT)
    energies = (E / AVG_N_ATOMS)[:, None] * E_SCALE \
        + meta["counts_ss"] @ W_comp.T
    return energies.astype(np.float32)


# ----------------------------------------------------------------------------
# Entry point
# ----------------------------------------------------------------------------

_PROGRAM_CACHE = {}


def kernel(ps, ln_gamma, ln_beta, W1, W2, W3, W_comp, species_idx, batch):
    from concourse import bass_utils

    in_maps, meta = host_prep(ps, ln_gamma, ln_beta, W1, W2, W3, W_comp,
                              species_idx, batch)
    key = (meta["T"], meta["species_map"], meta["scale1"], meta["scale2"])
    if key not in _PROGRAM_CACHE:
        _PROGRAM_CACHE[key] = build_program(meta["T"], meta["species_map"],
                                            meta["scale1"], meta["scale2"])
    nc = _PROGRAM_CACHE[key]
    res = bass_utils.run_bass_kernel_spmd(nc, in_maps,
                                          core_ids=list(range(N_CORES)))
    return aggregate(res.results, meta, batch, species_idx, W_comp)
